# revision 11
# baseline (speedup 1.0000x reference)
# Self-contained Trainium2 Bass kernel for the 2-layer dense GAT problem.
# kernel(**inputs) takes FULL inputs, shards rows across 8 NeuronCores,
# runs one SPMD Bass program (with a mid-kernel AllGather), returns FULL output.
import os
import sys

os.environ.setdefault("JAX_PLATFORMS", "axon")
sys.path.insert(0, "/opt/trn_rl_repo")

import numpy as np
import ml_dtypes

import concourse.bass as bass
import concourse.bacc as bacc
import concourse.tile as tile
from concourse import mybir
from concourse.masks import make_identity

BF16 = ml_dtypes.bfloat16
F32 = mybir.dt.float32
BF = mybir.dt.bfloat16

N, FEAT, H, O, OUT = 4096, 512, 8, 64, 512
NC = 8
RB = N // NC          # 512 rows per core
JT = N // 128         # 32 j tiles
KT = FEAT // 128      # 4 feature tiles
IT = RB // 128        # 4 local row tiles
ALPHA = 0.2
BIG = 1.0e38

AL = mybir.AluOpType
AF = mybir.ActivationFunctionType


def _bc(ap, n):
    """Broadcast an AP along a new innermost free dim of size n (step 0)."""
    return bass.AP(tensor=ap.tensor, offset=ap.offset, ap=list(ap.ap) + [[0, n]])


def build_program():
    nc = bacc.Bacc("TRN2", target_bir_lowering=False, debug=False, num_devices=NC)

    # ---------------- DRAM I/O ----------------
    d_xT = nc.dram_tensor("xT", [FEAT, N], BF, kind="ExternalInput")          # replicated
    d_xlocT = nc.dram_tensor("xlocT", [FEAT, RB], BF, kind="ExternalInput")   # per-core
    d_a01 = nc.dram_tensor("a01T", [N, RB], BF, kind="ExternalInput")         # per-core
    d_w1 = nc.dram_tensor("w1all", [FEAT, H * O], BF, kind="ExternalInput")
    d_waux = nc.dram_tensor("wauxall", [FEAT, 16], BF, kind="ExternalInput")  # [dst8|src8]
    d_wo = nc.dram_tensor("woall", [FEAT, OUT], BF, kind="ExternalInput")
    d_w2aux = nc.dram_tensor("w2aux", [FEAT, 2], BF, kind="ExternalInput")    # [dst|src]
    d_l1w = nc.dram_tensor("l1w", [OUT, 1024], BF, kind="ExternalInput")
    d_l1b = nc.dram_tensor("l1b", [1, 1024], BF, kind="ExternalInput")
    d_l2w = nc.dram_tensor("l2w", [1024, OUT], BF, kind="ExternalInput")
    d_l2b = nc.dram_tensor("l2b", [1, OUT], BF, kind="ExternalInput")
    d_out = nc.dram_tensor("out", [RB, OUT], F32, kind="ExternalOutput")

    # internal DRAM scratch
    d_srcrow = nc.dram_tensor("srcrow_d", [H, RB], BF)
    d_s2row = nc.dram_tensor("s2row_d", [1, RB], BF)
    d_kmax = nc.dram_tensor("kmax_d", [1, 1], F32)
    d_ccin = nc.dram_tensor("ccin_d", [RB, RB], BF)
    d_ccout = nc.dram_tensor("ccout_d", [N, RB], BF, addr_space="Shared")
    d_cc2in = nc.dram_tensor("cc2in_d", [1, RB], F32)
    d_cc2out = nc.dram_tensor("cc2out_d", [NC, RB], F32, addr_space="Shared")

    with tile.TileContext(nc) as tc:
        import contextlib
        ctx = contextlib.ExitStack()
        with ctx:
            consts = ctx.enter_context(tc.tile_pool(name="consts", bufs=1))

            ident = consts.tile([128, 128], BF)
            make_identity(nc, ident)
            ones_row = consts.tile([1, RB], BF)
            nc.vector.memset(ones_row, 1.0)

            # ---------------- weights into SBUF ----------------
            w1_sb = consts.tile([128, KT, H * O], BF)
            nc.sync.dma_start(w1_sb, d_w1.ap().rearrange("(k p) o -> p k o", p=128))
            waux_sb = consts.tile([128, KT, 16], BF)
            nc.sync.dma_start(waux_sb, d_waux.ap().rearrange("(k p) o -> p k o", p=128))
            wo_sb = consts.tile([128, KT, OUT], BF)
            nc.sync.dma_start(wo_sb, d_wo.ap().rearrange("(k p) o -> p k o", p=128))
            w2aux_sb = consts.tile([128, KT, 2], BF)
            nc.sync.dma_start(w2aux_sb, d_w2aux.ap().rearrange("(k p) o -> p k o", p=128))
            l1w_sb = consts.tile([128, KT, 1024], BF)
            nc.sync.dma_start(l1w_sb, d_l1w.ap().rearrange("(k p) o -> p k o", p=128))
            l1b_sb = consts.tile([1, 1024], BF)
            nc.sync.dma_start(l1b_sb, d_l1b.ap())
            l2w_sb = consts.tile([128, 8, OUT], BF)
            nc.sync.dma_start(l2w_sb, d_l2w.ap().rearrange("(k p) o -> p k o", p=128))
            l2b_sb = consts.tile([1, OUT], BF)
            nc.sync.dma_start(l2b_sb, d_l2b.ap())

            # masks: a01 lives through both layers; athr only through layer 1
            p_a01 = ctx.enter_context(tc.tile_pool(name="p_a01", bufs=1))
            a01_sb = p_a01.tile([128, JT, RB], BF)
            nc.sync.dma_start(a01_sb, d_a01.ap().rearrange("(t p) i -> p t i", p=128))
            p_h1 = ctx.enter_context(tc.tile_pool(name="p_h1", bufs=1))
            import contextlib as _cl
            l1ctx = _cl.ExitStack()
            big1 = l1ctx.enter_context(tc.tile_pool(name="big1", bufs=1))

            # ---------------- Phase W: Wh1 + aux projections ----------------
            whaug = big1.tile([128, JT, H, O + 1], BF)   # per-j Wh per head + ones col
            sdvec = big1.tile([128, JT, 16], F32)        # dst/src projections, all j
            sdloc = big1.tile([128, IT, 16], F32)        # projections at local rows
            srcrow_sb = consts.tile([H, RB], BF)

            with tc.tile_pool(name="xt_pool", bufs=1) as xtp, \
                 tc.tile_pool(name="wps", bufs=2, space="PSUM") as wps:
                xT_sb = xtp.tile([128, KT, N], BF)
                nc.sync.dma_start(xT_sb, d_xT.ap().rearrange("(k p) n -> p k n", p=128))
                xlocT_sb = xtp.tile([128, KT, RB], BF)
                nc.sync.dma_start(xlocT_sb, d_xlocT.ap().rearrange("(k p) i -> p k i", p=128))

                for jt in range(JT):
                    ps_wh = wps.tile([128, H * O], F32)
                    ps_aux = wps.tile([128, 16], F32, tag="aux")
                    for kt in range(KT):
                        lhsT = xT_sb[:, kt, jt * 128:(jt + 1) * 128]
                        nc.tensor.matmul(ps_wh, lhsT, w1_sb[:, kt, :],
                                         start=(kt == 0), stop=(kt == KT - 1))
                        nc.tensor.matmul(ps_aux, lhsT, waux_sb[:, kt, :],
                                         start=(kt == 0), stop=(kt == KT - 1))
                    nc.scalar.copy(whaug[:, jt, :, 0:O],
                                   ps_wh.rearrange("p (h o) -> p h o", h=H))
                    nc.vector.tensor_copy(sdvec[:, jt, :], ps_aux)

                # local src/dst (partition-major over local rows)
                for it in range(IT):
                    ps_loc = wps.tile([128, 16], F32, tag="aux")
                    for kt in range(KT):
                        nc.tensor.matmul(ps_loc, xlocT_sb[:, kt, it * 128:(it + 1) * 128],
                                         waux_sb[:, kt, :],
                                         start=(kt == 0), stop=(kt == KT - 1))
                    nc.vector.tensor_copy(sdloc[:, it, :], ps_loc)

                # src as rows: [H, RB] = w_src.T @ xloc
                ps_row = wps.tile([H, RB], F32, tag="aux")
                for kt in range(KT):
                    nc.tensor.matmul(ps_row, waux_sb[:, kt, 8:16], xlocT_sb[:, kt, :],
                                     start=(kt == 0), stop=(kt == KT - 1))
                nc.vector.tensor_copy(srcrow_sb, ps_row)
                nc.sync.dma_start(d_srcrow.ap(), srcrow_sb)

            nc.vector.memset(whaug[:, :, :, O], 1.0)  # ones column

            # exp of projections
            vexp_b = big1.tile([128, JT, H], BF)
            vpexp_b = big1.tile([128, JT, H], BF)
            nc.scalar.activation(vexp_b, sdvec[:, :, 0:8], AF.Exp)
            nc.scalar.activation(vpexp_b, sdvec[:, :, 0:8], AF.Exp, scale=ALPHA)
            uexp = big1.tile([128, IT, H], F32)
            upexp = big1.tile([128, IT, H], F32)
            nc.scalar.activation(uexp, sdloc[:, :, 8:16], AF.Exp)
            nc.scalar.activation(upexp, sdloc[:, :, 8:16], AF.Exp, scale=ALPHA)

            # src rows broadcast to all partitions, per head
            srcb = big1.tile([128, H, RB], BF)
            for h in range(H):
                nc.sync.dma_start(srcb[:, h, :],
                                  d_srcrow.ap()[h:h + 1, :].to_broadcast([128, RB]))

            # ---------------- Phase A: layer-1 aggregation ----------------
            h1sb = p_h1.tile([128, IT, H * O], BF)   # elu'd layer-1 out, feat = h*64+o

            with tc.tile_pool(name="vwh_pool", bufs=2) as vwhp, \
                 tc.tile_pool(name="t_pool", bufs=6) as tpool, \
                 tc.tile_pool(name="comb", bufs=2) as comb, \
                 tc.tile_pool(name="aps", bufs=1, space="PSUM") as aps:
                for h in range(H):
                    vwh2 = vwhp.tile([128, JT, 2 * (O + 1)], BF)
                    wh_h = whaug[:, :, h, :]                      # [128, JT, 65]
                    nc.vector.tensor_tensor(vwh2[:, :, 0:O + 1], wh_h,
                                            _bc(vexp_b[:, :, h], O + 1), op=AL.mult)
                    nc.vector.tensor_tensor(vwh2[:, :, O + 1:], wh_h,
                                            _bc(vpexp_b[:, :, h], O + 1), op=AL.mult)

                    ps_t = [aps.tile([128, 2 * (O + 1)], F32, tag=f"pst{it}", name=f"ps_t{it}")
                            for it in range(IT)]
                    ps_a = [aps.tile([128, O + 1], F32, tag=f"psa{it}", name=f"ps_a{it}")
                            for it in range(IT)]
                    for jt in range(JT):
                        tt0 = tpool.tile([128, RB], BF, tag="tt0")
                        nc.vector.tensor_scalar(tt0, srcb[:, h, :],
                                                sdvec[:, jt, h:h + 1], 0.0,
                                                op0=AL.add, op1=AL.is_ge)
                        tt = tpool.tile([128, RB], BF)
                        eng = nc.gpsimd if h < 5 else nc.vector
                        eng.tensor_tensor(tt, tt0, a01_sb[:, jt, :], op=AL.mult)
                        st, sp = (jt == 0), (jt == JT - 1)
                        for it in range(IT):
                            tsl = tt[:, it * 128:(it + 1) * 128]
                            nc.tensor.matmul(ps_t[it], tsl, vwh2[:, jt, :],
                                             start=st, stop=sp)
                            nc.tensor.matmul(ps_a[it],
                                             a01_sb[:, jt, it * 128:(it + 1) * 128],
                                             vwh2[:, jt, O + 1:], start=st, stop=sp)

                    # stage psum -> sbuf, then batched combine over all 4 it
                    qsb = comb.tile([128, IT, 2 * (O + 1)], F32, tag="qsb")
                    for it in range(IT):
                        nc.vector.tensor_copy(qsb[:, it, :], ps_t[it])
                    # num = u*t1 + u'*(t2 - t3); den at col O
                    asb = comb.tile([128, IT, O + 1], F32, tag="asb")
                    for it in range(IT):
                        nc.vector.tensor_copy(asb[:, it, :], ps_a[it])
                    diff = comb.tile([128, IT, O + 1], F32, tag="diff")
                    nc.vector.tensor_tensor(diff, asb, qsb[:, :, O + 1:], op=AL.subtract)
                    q1 = comb.tile([128, IT, O + 1], F32, tag="q1")
                    nc.vector.tensor_tensor(q1, qsb[:, :, 0:O + 1],
                                            _bc(uexp[:, :, h], O + 1), op=AL.mult)
                    nc.vector.tensor_tensor(diff, diff,
                                            _bc(upexp[:, :, h], O + 1), op=AL.mult)
                    nc.vector.tensor_tensor(q1, q1, diff, op=AL.add)

                    rec = comb.tile([128, IT, 1], F32, tag="rec")
                    nc.vector.reciprocal(rec, q1[:, :, O:O + 1])
                    h1r = comb.tile([128, IT, O], F32, tag="h1r")
                    nc.vector.tensor_tensor(h1r, q1[:, :, 0:O],
                                            _bc(rec[:, :, 0], O), op=AL.mult)
                    # elu
                    ex = comb.tile([128, IT, O], F32, tag="ex")
                    nc.scalar.activation(ex, h1r, AF.Exp)
                    nc.vector.tensor_scalar(ex, ex, 1.0, -1.0, op0=AL.min, op1=AL.add)
                    nc.vector.scalar_tensor_tensor(
                        h1sb[:, :, h * O:(h + 1) * O], in0=h1r, scalar=0.0, in1=ex,
                        op0=AL.max, op1=AL.add)

            # ---------------- transpose h1 -> [feat, local] and AllGather ----------------
            h1T = p_h1.tile([128, IT, RB], BF)
            with tc.tile_pool(name="tp_ps", bufs=2, space="PSUM") as tps:
                for ft in range(IT):
                    for it in range(IT):
                        pst = tps.tile([128, 128], BF)
                        nc.tensor.transpose(pst, h1sb[:, it, ft * 128:(ft + 1) * 128], ident)
                        nc.vector.tensor_copy(h1T[:, ft, it * 128:(it + 1) * 128], pst)
            # local layer-2 projections from h1T (before the gathers)
            src2row = consts.tile([1, RB], F32)
            dst2loc = consts.tile([1, RB], F32)
            with tc.tile_pool(name="prew", bufs=2, space="PSUM") as prew:
                ps_r2 = prew.tile([1, RB], F32)
                ps_d2 = prew.tile([1, RB], F32, tag="d2")
                for kt in range(KT):
                    nc.tensor.matmul(ps_r2, w2aux_sb[:, kt, 1:2], h1T[:, kt, :],
                                     start=(kt == 0), stop=(kt == KT - 1))
                    nc.tensor.matmul(ps_d2, w2aux_sb[:, kt, 0:1], h1T[:, kt, :],
                                     start=(kt == 0), stop=(kt == KT - 1))
                nc.vector.tensor_copy(src2row, ps_r2)
                nc.vector.tensor_copy(dst2loc, ps_d2)

            s2shift = consts.tile([1, RB], BF)
            nc.vector.tensor_copy(s2shift, src2row)
            nc.sync.dma_start(d_s2row.ap(), s2shift)
            nc.sync.dma_start(d_cc2in.ap(), dst2loc)

            nc.sync.dma_start(d_ccin.ap().rearrange("(f p) i -> p f i", p=128), h1T)
            nc.gpsimd.collective_compute(
                "AllGather", AL.bypass, ins=[d_cc2in.ap().opt()],
                outs=[d_cc2out.ap().opt()], replica_groups=[list(range(NC))])
            nc.gpsimd.collective_compute(
                "AllGather", AL.bypass, ins=[d_ccin.ap().opt()], outs=[d_ccout.ap().opt()],
                replica_groups=[list(range(NC))])

            # ---------------- during big gather: kmax, masks e2 for all jt ----------
            l1ctx.close()
            big2 = ctx.enter_context(tc.tile_pool(name="big2", bufs=1))
            src2b = big2.tile([128, RB], BF)
            nc.sync.dma_start(src2b, d_s2row.ap()[0:1, :].to_broadcast([128, RB]))
            dst2s = big2.tile([128, JT, 1], F32)
            nc.sync.dma_start(dst2s[:, :, 0],
                              d_cc2out.ap().rearrange("c (t p) -> p (c t)", p=128))

            e2sb = big2.tile([128, JT, RB], BF)
            with tc.tile_pool(name="e2_pool", bufs=2) as e2p:
                for jt in range(JT):
                    s2t = e2p.tile([128, RB], F32, tag="s2t")
                    nc.scalar.activation(s2t, src2b, AF.Identity, bias=dst2s[:, jt, 0:1])
                    ls = e2p.tile([128, RB], F32, tag="ls")
                    nc.vector.scalar_tensor_tensor(ls, in0=s2t, scalar=ALPHA, in1=s2t,
                                                   op0=AL.mult, op1=AL.max)
                    nc.scalar.activation(e2sb[:, jt, :], ls, AF.Exp)
                    nc.vector.tensor_tensor(e2sb[:, jt, :], e2sb[:, jt, :],
                                            a01_sb[:, jt, :], op=AL.mult)

            # ---------------- Phase W2 + A2 fused over jt ----------------
            x1t_sb = big2.tile([128, JT, RB], BF)     # x1^T tiles: row (c,f), col i
            nc.sync.dma_start(x1t_sb, d_ccout.ap().rearrange("(t p) i -> p t i", p=128))

            wh2aug = big2.tile([128, JT, OUT + 1], BF)
            nc.vector.memset(wh2aug[:, :, OUT], 1.0)

            o2sb = big2.tile([128, IT, OUT], BF)
            with tc.tile_pool(name="w2ps", bufs=2, space="PSUM") as w2ps, \
                 tc.tile_pool(name="a2ps", bufs=1, space="PSUM") as a2ps, \
                 tc.tile_pool(name="c2", bufs=2) as c2:
                ps_o2 = a2ps.tile([128, IT, OUT], F32)
                ps_dn = a2ps.tile([128, IT], F32, tag="psdn")
                for jt in range(JT):
                    ps_wh2 = w2ps.tile([128, OUT], F32)
                    c = jt // IT
                    i0 = (jt % IT) * 128
                    for kt in range(KT):
                        lhsT = x1t_sb[:, c * KT + kt, i0:i0 + 128]
                        nc.tensor.matmul(ps_wh2, lhsT, wo_sb[:, kt, :],
                                         start=(kt == 0), stop=(kt == KT - 1))
                    nc.scalar.copy(wh2aug[:, jt, 0:OUT], ps_wh2)
                    st, sp = (jt == 0), (jt == JT - 1)
                    for it in range(IT):
                        esl = e2sb[:, jt, it * 128:(it + 1) * 128]
                        nc.tensor.matmul(ps_o2[:, it, :], esl, wh2aug[:, jt, 0:OUT],
                                         start=st, stop=sp)
                        nc.tensor.matmul(ps_dn[:, it:it + 1], esl,
                                         wh2aug[:, jt, OUT:OUT + 1],
                                         start=st, stop=sp)
                dnsb = c2.tile([128, IT, 1], F32)
                nc.vector.tensor_copy(dnsb[:, :, 0], ps_dn)
                rec2 = c2.tile([128, IT, 1], F32)
                nc.vector.reciprocal(rec2, dnsb)
                for it in range(IT):
                    o2r = c2.tile([128, OUT], F32, tag="o2r")
                    nc.vector.tensor_tensor(o2r, ps_o2[:, it, :],
                                            _bc(rec2[:, it, 0], OUT), op=AL.mult)
                    ex2 = c2.tile([128, OUT], F32, tag="ex2")
                    nc.scalar.activation(ex2, o2r, AF.Exp)
                    nc.vector.tensor_scalar(ex2, ex2, 1.0, -1.0, op0=AL.min, op1=AL.add)
                    nc.vector.scalar_tensor_tensor(o2sb[:, it, :], in0=o2r, scalar=0.0,
                                                   in1=ex2, op0=AL.max, op1=AL.add)

            # ---------------- transpose o2 -> o2T, lin1 + lin2 ----------------
            o2T = big2.tile([128, IT, RB], BF)
            with tc.tile_pool(name="tp2_ps", bufs=2, space="PSUM") as tps2:
                for ft in range(IT):
                    for it in range(IT):
                        pst = tps2.tile([128, 128], BF)
                        nc.tensor.transpose(pst, o2sb[:, it, ft * 128:(ft + 1) * 128], ident)
                        nc.vector.tensor_copy(o2T[:, ft, it * 128:(it + 1) * 128], pst)

            o3T = big2.tile([128, 8, RB], BF)
            out_sb = big2.tile([128, IT, OUT], F32)
            with tc.tile_pool(name="l_ps", bufs=4, space="PSUM") as lps:
                for mq in range(8):
                    ps = lps.tile([128, RB], F32)
                    for kt in range(KT):
                        nc.tensor.matmul(ps, l1w_sb[:, kt, mq * 128:(mq + 1) * 128],
                                         o2T[:, kt, :], start=(kt == 0), stop=False)
                    nc.tensor.matmul(ps, l1b_sb[:, mq * 128:(mq + 1) * 128], ones_row,
                                     start=False, stop=True)
                    nc.scalar.activation(o3T[:, mq, :], ps, AF.Relu)

                for mi in range(IT):
                    ps = lps.tile([128, OUT], F32)
                    for kq in range(8):
                        nc.tensor.matmul(ps, o3T[:, kq, mi * 128:(mi + 1) * 128],
                                         l2w_sb[:, kq, :], start=(kq == 0), stop=False)
                    nc.tensor.matmul(ps, ones_row[:, 0:128], l2b_sb,
                                     start=False, stop=True)
                    nc.vector.tensor_copy(out_sb[:, mi, :], ps)

            nc.sync.dma_start(d_out.ap().rearrange("(t p) o -> p t o", p=128), out_sb)

    nc.compile()
    return nc


_CACHE = {}


def _prep_inputs(inputs):
    x = np.asarray(inputs["x"], np.float32)
    adj = np.asarray(inputs["adj"])
    W1 = np.asarray(inputs["W1"], np.float32)
    a1 = np.asarray(inputs["a1"], np.float32)
    Wo = np.asarray(inputs["Wo"], np.float32)
    ao = np.asarray(inputs["ao"], np.float32)
    l1w = np.asarray(inputs["lin1_w"], np.float32)
    l1b = np.asarray(inputs["lin1_b"], np.float32)
    l2w = np.asarray(inputs["lin2_w"], np.float32)
    l2b = np.asarray(inputs["lin2_b"], np.float32)

    xT = np.ascontiguousarray(x.T).astype(BF16)
    w_dst = np.einsum("hfo,ho->fh", W1, a1[:, O:]).astype(np.float32)   # [F, H]
    w_src = np.einsum("hfo,ho->fh", W1, a1[:, :O]).astype(np.float32)
    wauxall = np.ascontiguousarray(
        np.concatenate([w_dst, w_src], axis=1)).astype(BF16)            # [F, 16]
    w1all = np.ascontiguousarray(W1.transpose(1, 0, 2).reshape(FEAT, H * O)).astype(BF16)
    w2aux = np.ascontiguousarray(
        np.stack([Wo @ ao[OUT:], Wo @ ao[:OUT]], axis=1)).astype(BF16)  # [F, 2]

    rep = dict(
        xT=xT, w1all=w1all, wauxall=wauxall, woall=np.ascontiguousarray(Wo).astype(BF16),
        w2aux=w2aux,
        l1w=np.ascontiguousarray(l1w).astype(BF16),
        l1b=np.ascontiguousarray(l1b.reshape(1, -1)).astype(BF16),
        l2w=np.ascontiguousarray(l2w).astype(BF16),
        l2b=np.ascontiguousarray(l2b.reshape(1, -1)).astype(BF16),
    )
    in_maps = []
    for c in range(NC):
        rows = slice(c * RB, (c + 1) * RB)
        ablk = adj[rows, :] > 0                       # [RB, N]
        a01T = ablk.T.astype(BF16)
        m = dict(rep)
        m["xlocT"] = np.ascontiguousarray(x[rows, :].T).astype(BF16)
        m["a01T"] = np.ascontiguousarray(a01T)
        in_maps.append(m)
    return in_maps


def kernel(**inputs):
    from concourse.bass_utils import run_bass_kernel_spmd

    if "nc" not in _CACHE:
        _CACHE["nc"] = build_program()
    nc = _CACHE["nc"]

    in_maps = _prep_inputs(inputs)
    trace = bool(_CACHE.get("trace"))
    res = run_bass_kernel_spmd(nc, in_maps, core_ids=list(range(NC)), trace=trace)
    _CACHE["last_results"] = res
    out = np.concatenate([r["out"] for r in res.results], axis=0)
    return out.astype(np.float32)

